# revision 7
# baseline (speedup 1.0000x reference)
"""nGPT-style cosine-norm attention on 8 TRN2 NeuronCores, data-parallel over batch.

Per core (one batch element, tokens N=1024, dim 768, 12 heads x 64):
  qT/kT = WT_eff @ xT  (head-dim on partitions), v in token-major layout (+ones col)
  ss    = blockdiag(1/s_eff^2) @ (qT^2)  -> per-head token norms via PE
  rq,rk = Newton-refined rsqrt;  qn = qT * bcast(rq) (DMA row-broadcast)
  S^T   = kn_h^T q_h  per (head, jtile);  E = exp(8*rk_j * S^T) (ACT per-partition scale)
  PV    = [V_h | 1]^T E  -> attention rows + denominator row in one accumulation
  attn  = PV[0:64]; normalize by bcast(1/D) after all heads; out = attn^T @ WoT
All matmuls bf16 (inputs pre-cast on host), stats/softmax f32.
"""
import json
import math

import numpy as np
import ml_dtypes

B, N, DIM, H, HD = 8, 1024, 768, 12, 64
P = 128
CH = DIM // P  # 6 chunks of 128 rows; chunk c holds heads 2c, 2c+1
SCALE = float(math.sqrt(HD))
BF = ml_dtypes.bfloat16

_cache = {}
SKIP = set()  # debug: bisect compile failures


def _split_waits(nc, cap=1):
    """This walrus build caps sync-waits per instruction (1 for several structs).
    Move excess waits onto NoOps inserted immediately before, same engine."""
    from bass_rust import module_from_json_bytes

    js = json.loads(nc.to_json_bytes())
    ctr = 0
    for f in js["functions"]:
        for bb in f["blocks"]:
            newl = []
            for inst in bb["instructions"]:
                si = inst.get("sync_info")
                waits = (si or {}).get("on_wait") or []
                if len(waits) > cap:
                    extra, keep = waits[:-cap], waits[-cap:]
                    for k in range(0, len(extra), cap):
                        ctr += 1
                        newl.append({
                            "debug": inst.get("debug", 0),
                            "engine": inst["engine"],
                            "ins": [], "outs": [],
                            "name": f"wsplit-{ctr}",
                            "opcode": "NoOp",
                            "sync_info": {"on_update": [],
                                          "on_wait": extra[k:k + cap]},
                        })
                    si["on_wait"] = keep
                newl.append(inst)
            bb["instructions"] = newl
    nc.m = module_from_json_bytes(json.dumps(js).encode())


def build_nc():
    import concourse.bass as bass
    import concourse.tile as tile
    from concourse import mybir

    f32 = mybir.dt.float32
    bf16 = mybir.dt.bfloat16
    Exp = mybir.ActivationFunctionType.Exp
    Sqrt = mybir.ActivationFunctionType.Sqrt
    mult = mybir.AluOpType.mult
    add = mybir.AluOpType.add

    nc = bass.Bass("TRN2", num_devices=8)
    xT_d = nc.dram_tensor("xT", [DIM, N], bf16, kind="ExternalInput")
    wq_d = nc.dram_tensor("wq", [DIM, DIM], bf16, kind="ExternalInput")
    wk_d = nc.dram_tensor("wk", [DIM, DIM], bf16, kind="ExternalInput")
    wv_d = nc.dram_tensor("wv", [DIM, DIM], bf16, kind="ExternalInput")
    wo_d = nc.dram_tensor("wo", [DIM, DIM], bf16, kind="ExternalInput")
    invs2_d = nc.dram_tensor("invs2", [P, CH * H], bf16, kind="ExternalInput")
    ident_d = nc.dram_tensor("ident", [H, H], f32, kind="ExternalInput")
    out_d = nc.dram_tensor("out", [N, DIM], f32, kind="ExternalOutput")

    with tile.TileContext(nc) as tc:
        with (
            tc.tile_pool(name="persist", bufs=1) as pp,
            tc.tile_pool(name="dram", bufs=1, space="DRAM") as dp,
            tc.tile_pool(name="epool", bufs=4) as ep,
            tc.tile_pool(name="bcast", bufs=2) as bcp,
            tc.tile_pool(name="small", bufs=1) as smp,
            tc.tile_pool(name="sqp", bufs=2) as sqp,
            tc.tile_pool(name="dtp", bufs=2) as dtp,
            tc.tile_pool(name="outp", bufs=2) as outp,
        ):
            # ---- persistent SBUF ----
            xT = pp.tile([P, CH, N], bf16)
            wq = pp.tile([P, CH, DIM], bf16)
            wk = pp.tile([P, CH, DIM], bf16)
            wv = pp.tile([P, CH, DIM], bf16)
            wo = pp.tile([P, CH, DIM], bf16)
            invs2 = pp.tile([P, CH, H], bf16)
            ident = pp.tile([H, H], f32)
            qT = pp.tile([P, CH, N], bf16)
            kT = pp.tile([P, CH, N], bf16)
            v1 = pp.tile([P, 8, H, HD + 1], bf16)
            attn = pp.tile([P, CH, N], bf16)
            rkT = pp.tile([P, 8, H], f32)

            nc.sync.dma_start(out=xT, in_=xT_d[:, :].rearrange("(c p) n -> p c n", p=P))
            nc.sync.dma_start(out=wq, in_=wq_d[:, :].rearrange("(c p) o -> p c o", p=P))
            nc.sync.dma_start(out=wk, in_=wk_d[:, :].rearrange("(c p) o -> p c o", p=P))
            nc.sync.dma_start(out=wv, in_=wv_d[:, :].rearrange("(c p) o -> p c o", p=P))
            nc.sync.dma_start(out=wo, in_=wo_d[:, :].rearrange("(c p) o -> p c o", p=P))
            nc.sync.dma_start(out=invs2, in_=invs2_d[:, :].rearrange("p (c h) -> p c h", h=H))
            nc.sync.dma_start(out=ident, in_=ident_d[:, :])

            # DRAM scratch
            rq_dram = dp.tile([H, N], bf16)
            d_dram = dp.tile([H, N], f32)
            rd_dram = dp.tile([H, N], bf16)

            # ================= Phase 1: projections + norms =================
            with (
                tc.tile_pool(name="projps", bufs=2, space="PSUM") as pjp,
                tc.tile_pool(name="ssps", bufs=2, space="PSUM") as ssp,
            ):
                # v projection (token-major); whole tile preset to 1.0 so the
                # per-head 65th column acts as the denominator ones-column
                nc.vector.memset(v1[:, :, :, :], 1.0)
                for m in range(8):
                    ps = pjp.tile([P, DIM], f32, tag="proj")
                    for o2 in range(2):
                        osl = slice(o2 * 512, min(DIM, (o2 + 1) * 512))
                        for k in range(CH):
                            nc.tensor.matmul(
                                ps[:, osl],
                                xT[:, k, m * P:(m + 1) * P],
                                wv[:, k, osl],
                                start=(k == 0), stop=(k == CH - 1),
                            )
                    if "v1copy" in SKIP:
                        nc.vector.tensor_copy(out=v1[:, m, 0, 0:HD], in_=ps[:, 0:HD])
                    else:
                        nc.vector.tensor_copy(
                            out=v1[:, m, :, 0:HD],
                            in_=ps[:, :].rearrange("p (h d) -> p h d", d=HD),
                        )

                # q/k projections (head-dim major)
                for dst, w in ((qT, wq), (kT, wk)):
                    for c in range(CH):
                        for n2 in range(2):
                            nsl = slice(n2 * 512, (n2 + 1) * 512)
                            ps = pjp.tile([P, DIM], f32, tag="proj")
                            for k in range(CH):
                                nc.tensor.matmul(
                                    ps[:, 0:512],
                                    w[:, k, c * P:(c + 1) * P],
                                    xT[:, k, nsl],
                                    start=(k == 0), stop=(k == CH - 1),
                                )
                            nc.vector.tensor_copy(out=dst[:, c, nsl], in_=ps[:, 0:512])

                # per-head squared norms: ss[h, i] = sum_d (q[d,i]/s_d)^2
                stats = []
                for src in (qT, kT):
                    ssq = ssp.tile([H, N], f32, tag="ss")
                    for c in range(CH):
                        sq = sqp.tile([P, N], bf16, tag="sq")
                        nc.vector.tensor_tensor(sq, src[:, c, :], src[:, c, :], mult)
                        for n2 in range(2):
                            nsl = slice(n2 * 512, (n2 + 1) * 512)
                            nc.tensor.matmul(
                                ssq[:, nsl], invs2[:, c, :], sq[:, nsl],
                                start=(c == 0), stop=(c == CH - 1),
                            )
                    stats.append(ssq)

                # rq/rk = 1/sqrt(ss) with one Newton step (fixes Sqrt table err)
                eps = smp.tile([H, 1], f32, tag="eps")
                nc.vector.memset(eps, 1e-12)
                routs = []
                for i, ssq in enumerate(stats):
                    s = smp.tile([H, N], f32, tag="st1")
                    nc.scalar.activation(out=s, in_=ssq, func=Sqrt, bias=eps[:, 0:1])
                    r0 = smp.tile([H, N], f32, tag="st2")
                    nc.vector.reciprocal(out=r0, in_=s)
                    t1 = smp.tile([H, N], f32, tag="st3")
                    nc.vector.tensor_tensor(t1, ssq, r0, mult)
                    t2 = smp.tile([H, N], f32, tag="st4")
                    nc.vector.tensor_tensor(t2, t1, r0, mult)
                    t3 = smp.tile([H, N], f32, tag="st5")
                    if i == 0:  # q: r = r0*(1.5 - 0.5*t2)
                        nc.vector.tensor_scalar(t3, t2, -0.5, 1.5, op0=mult, op1=add)
                        r = smp.tile([H, N], bf16, tag="st6")
                    else:  # k: fold logit scale 8: r = 8*r0*(1.5 - 0.5*t2)
                        nc.vector.tensor_scalar(t3, t2, -0.5 * SCALE, 1.5 * SCALE,
                                                op0=mult, op1=add)
                        r = smp.tile([H, N], f32, tag="st7")
                    nc.vector.tensor_tensor(r, t3, r0, mult)
                    routs.append(r)
                rq_bf, rk_f = routs

                # rq: bf16 -> DRAM (for row-broadcast reads)
                nc.sync.dma_start(out=rq_dram[:, :], in_=rq_bf)
                # rk: transpose to token-major [128, jt, h] via PE
                if "transpose" in SKIP:
                    nc.vector.memset(rkT[:, :, :], 1.0)
                else:
                    for jt in range(8):
                        tp = ssp.tile([P, H], f32, tag="ss")
                        nc.tensor.transpose(tp, rk_f[:, jt * P:(jt + 1) * P], ident)
                        nc.vector.tensor_copy(out=rkT[:, jt, :], in_=tp)

                # qn = qT * bcast(rq) per chunk
                for c in range(CH):
                    mq = bcp.tile([P, N], bf16, tag="mq")
                    if "bcast" in SKIP:
                        nc.vector.memset(mq[:, :], 1.0)
                    else:
                        for hh in range(2):
                            row = rq_dram[2 * c + hh:2 * c + hh + 1, :]
                            bc = bass.AP(tensor=row.tensor, offset=row.offset,
                                         ap=[[0, 64]] + list(row.ap[1:]))
                            nc.sync.dma_start(out=mq[hh * 64:(hh + 1) * 64, :], in_=bc)
                    nc.vector.tensor_tensor(qT[:, c, :], qT[:, c, :], mq, mult)

            # ================= Phase 2: attention =================
            with (
                tc.tile_pool(name="sps", bufs=3, space="PSUM") as sps,
                tc.tile_pool(name="pvps", bufs=1, space="PSUM") as pvp,
            ):
                for h in range(H):
                    c, half = h // 2, (h % 2) * 64
                    pv = pvp.tile([HD + 1, N], f32, tag="pv")
                    for jt in range(8):
                        s = sps.tile([P, N], f32, tag="S")
                        for n2 in range(2):
                            nsl = slice(n2 * 512, (n2 + 1) * 512)
                            nc.tensor.matmul(
                                s[:, nsl],
                                kT[half:half + 64, c, jt * P:(jt + 1) * P],
                                qT[half:half + 64, c, nsl],
                                start=True, stop=True,
                            )
                        e = ep.tile([P, N], bf16, tag="E")
                        if "scaleap" in SKIP:
                            nc.scalar.activation(out=e, in_=s, func=Exp, scale=1.0)
                        else:
                            nc.scalar.activation(out=e, in_=s, func=Exp,
                                                 scale=rkT[:, jt, h:h + 1])
                        for n2 in range(2):
                            nsl = slice(n2 * 512, (n2 + 1) * 512)
                            nc.tensor.matmul(
                                pv[:, nsl], v1[:, jt, h, :], e[:, nsl],
                                start=(jt == 0), stop=(jt == 7),
                            )
                    # denominator row -> partition-0 tile -> DRAM row h
                    dtmp = dtp.tile([1, N], f32, tag="dt")
                    nc.vector.tensor_copy(out=dtmp, in_=pv[HD:HD + 1, :])
                    nc.sync.dma_start(out=d_dram[h:h + 1, :], in_=dtmp)
                    nc.vector.tensor_copy(out=attn[half:half + 64, c, :],
                                          in_=pv[0:HD, :])

                # spread denominators across partitions, reciprocal
                dall = smp.tile([H, N], f32, tag="st1")
                nc.sync.dma_start(out=dall, in_=d_dram[:, :])
                rd_f = smp.tile([H, N], f32, tag="st2")
                nc.vector.reciprocal(out=rd_f, in_=dall)
                rd_bf = smp.tile([H, N], bf16, tag="st6")
                nc.vector.tensor_copy(rd_bf, rd_f)
                nc.sync.dma_start(out=rd_dram[:, :], in_=rd_bf)

                for c in range(CH):
                    mr = bcp.tile([P, N], bf16, tag="mq")
                    if "bcast" in SKIP:
                        nc.vector.memset(mr[:, :], 1.0)
                    else:
                        for hh in range(2):
                            row = rd_dram[2 * c + hh:2 * c + hh + 1, :]
                            bc = bass.AP(tensor=row.tensor, offset=row.offset,
                                         ap=[[0, 64]] + list(row.ap[1:]))
                            nc.sync.dma_start(out=mr[hh * 64:(hh + 1) * 64, :], in_=bc)
                    nc.vector.tensor_tensor(attn[:, c, :], attn[:, c, :], mr, mult)

            # ================= Phase 3: output projection =================
            with tc.tile_pool(name="ops", bufs=2, space="PSUM") as opp:
                for m in range(8):
                    ps = opp.tile([P, DIM], f32, tag="out")
                    for o2 in range(2):
                        osl = slice(o2 * 512, min(DIM, (o2 + 1) * 512))
                        for c in range(CH):
                            nc.tensor.matmul(
                                ps[:, osl],
                                attn[:, c, m * P:(m + 1) * P],
                                wo[:, c, osl],
                                start=(c == 0), stop=(c == CH - 1),
                            )
                    osb = outp.tile([P, DIM], f32, tag="osb")
                    nc.vector.tensor_copy(osb, ps)
                    nc.sync.dma_start(out=out_d[m * P:(m + 1) * P, :], in_=osb)

    _split_waits(nc, cap=1)
    return nc


def _host_inputs(x, Wq, Wk, Wv, Wo, s_qk):
    s_eff = (np.asarray(s_qk, np.float32).reshape(-1) * math.sqrt(DIM)).astype(np.float32)
    wq = np.ascontiguousarray((s_eff[:, None] * np.asarray(Wq, np.float32)).T).astype(BF)
    wk = np.ascontiguousarray((s_eff[:, None] * np.asarray(Wk, np.float32)).T).astype(BF)
    wv = np.ascontiguousarray(np.asarray(Wv, np.float32).T).astype(BF)
    wo = np.ascontiguousarray(np.asarray(Wo, np.float32).T).astype(BF)
    invs2 = np.zeros((P, CH * H), np.float32)
    for o in range(DIM):
        c, p, h = o // P, o % P, o // HD
        invs2[p, c * H + h] = 1.0 / (s_eff[o] * s_eff[o])
    invs2 = invs2.astype(BF)
    ident = np.eye(H, dtype=np.float32)
    shared = dict(wq=wq, wk=wk, wv=wv, wo=wo, invs2=invs2, ident=ident)
    in_maps = []
    for b in range(B):
        m = dict(shared)
        m["xT"] = np.ascontiguousarray(np.asarray(x[b], np.float32).T).astype(BF)
        in_maps.append(m)
    return in_maps


def run(x, Wq, Wk, Wv, Wo, s_qk, trace=False, **trace_kwargs):
    from concourse.bass_utils import run_bass_kernel_spmd

    if "nc" not in _cache:
        _cache["nc"] = build_nc()
    nc = _cache["nc"]
    in_maps = _host_inputs(x, Wq, Wk, Wv, Wo, s_qk)
    res = run_bass_kernel_spmd(nc, in_maps, core_ids=list(range(8)),
                               trace=trace, **trace_kwargs)
    out = np.stack([res.results[b]["out"] for b in range(B)]).astype(np.float32)
    return out, res


def kernel(x, Wq, Wk, Wv, Wo, s_qk):
    out, _ = run(x, Wq, Wk, Wv, Wo, s_qk, trace=False)
    return out
